# revision 3
# baseline (speedup 1.0000x reference)
"""Bahdanau attention Trainium2 kernel.

  keys_proj = values @ W1 + b1            # (B, T, U)
  query_proj = query @ W2 + b2            # (B, 1, U)
  score = tanh(keys_proj + query_proj) @ V + bv   # (B, T, 1)
  aw = softmax(score, axis=1)             # (B, T, 1)
  ctx = sum(aw * values, axis=1)          # (B, D)

Sharding: data-parallel over batch, 4 batches per core on 8 cores. Each core
is fully independent (no collectives).

Per-core dataflow (per batch, values read from HBM exactly once):
  - values tiles loaded naturally [t=128, d=2048] (contiguous DMA), kept in
    SBUF for the whole batch (used twice: transpose source + context matmul).
  - PE-transpose 128x128 blocks -> vT [d=128, t] chunks.
  - kp[u=128, t] += W1[d,u-tile].T @ vT (PSUM accumulate over 16 d-tiles),
    float32r so the PE runs at full rate with 4-byte operands (moving dim 256).
  - ScalarE: th = tanh(kp + (q @ W2 + b2 + b1)[u]) with per-partition bias,
    PSUM -> SBUF.
  - score[1, t] += V[u-tile].T @ th  (PSUM accumulate over 8 u-tiles).
  - softmax on [1, T]: DVE max (negated), ACT exp with fused sum accumulator,
    DVE reciprocal + scale.  bv is dropped: softmax is shift-invariant and
    both outputs depend on score only through softmax.
  - aw transposed to [t=128, 8] via K=1 matmuls against ones.
  - ctx[1, d] += awT[:, c].T @ values_nat  (accumulate over 8 t-tiles).
"""

import os
from contextlib import ExitStack

import numpy as np

import concourse.bass as bass
import concourse.mybir as mybir
import concourse.tile as tile
from concourse.bass_utils import run_bass_kernel_spmd
from concourse.masks import make_identity

B, T, D, U = 32, 1024, 2048, 1024
NCORES = 8
BPC = B // NCORES  # batches per core
P = 128
DT = D // P  # 16 d-tiles
UT = U // P  # 8 u-tiles
TCH = 256  # t-chunk (moving dim of the main matmul; >=256 for f32r full rate)
NCH = T // TCH  # chunks per batch
TT = TCH // P  # t-subtiles per chunk
DCH = 512  # d-chunk of the context matmul
NAT_BUFS = 9  # T//P live nat tiles per batch + prefetch

F32 = mybir.dt.float32
F32R = mybir.dt.float32r
BF16 = mybir.dt.bfloat16
AF = mybir.ActivationFunctionType

# "f32r": fp32-width operands in replicated-fp32 PE mode (full PE rate).
# "bf16": main (values @ W1) matmul in bf16 instead.
KP_MODE = os.environ.get("BAHDANAU_KP_MODE", "f32r")


def _split_sync_waits(nc, max_waits=1):
    """walrus's CTRL lowering in this toolchain accepts only one sem wait per
    instruction; split any instruction carrying more into preceding bare
    Drain wait-carriers on the same engine."""
    n = 0
    for bb in nc.m.functions[0].blocks:
        insts = bb.instructions
        i = 0
        while i < len(insts):
            inst = insts[i]
            si = inst.sync_info
            if si is not None and si.on_wait and len(si.on_wait) > max_waits:
                waits = list(si.on_wait)
                extra, keep = waits[:-max_waits], waits[-max_waits:]
                carriers = []
                for j in range(0, len(extra), max_waits):
                    c = mybir.InstDrain(name=f"{inst.name}-ws{n}", ins=[], outs=[])
                    n += 1
                    c.engine = inst.engine
                    c.sync_info = mybir.SyncInfo(
                        on_wait=extra[j : j + max_waits], on_update=[]
                    )
                    nc.register_instruction(c, overwrite=True)
                    carriers.append(c)
                si.on_wait = keep
                insts[i:i] = carriers
                i += len(carriers)
            i += 1
    return n


def _kernel_body(ctx, tc, q, v, w1, b1, w2, b2, vv, ctx_o, aw_o):
    nc = tc.nc
    kp_bf16 = KP_MODE == "bf16"

    const = ctx.enter_context(tc.tile_pool(name="const", bufs=1))
    identity = const.tile([P, P], F32)
    make_identity(nc, identity[:])
    identr = const.tile([P, P], F32R)
    nc.vector.tensor_copy(out=identr[:], in_=identity[:])
    ones = const.tile([1, 1], F32)
    nc.vector.memset(ones[:], 1.0)

    vsb = const.tile([P, UT], F32R)  # V in [u mod 128, u-tile] layout
    nc.sync.dma_start(out=vsb[:], in_=vv.rearrange("(c p) o -> p (c o)", p=P))
    bias12 = const.tile([P, UT], F32)  # b1 + b2
    b1sb = const.tile([P, UT], F32)
    nc.sync.dma_start(out=b1sb[:], in_=b1.rearrange("(c p) -> p c", p=P))
    nc.sync.dma_start(out=bias12[:], in_=b2.rearrange("(c p) -> p c", p=P))
    nc.vector.tensor_add(bias12[:], bias12[:], b1sb[:])

    # qp[u, i, b] = (query @ W2 + b2 + b1) per u-tile, per-partition bias for tanh
    qp = const.tile([P, UT, BPC], F32)

    # ---- W1 resident for the whole kernel ----
    w1_dt = BF16 if kp_bf16 else F32R
    w1p = ctx.enter_context(tc.tile_pool(name="w1", bufs=1))
    w1t = w1p.tile([P, DT, U], w1_dt)

    if kp_bf16:
        with tc.tile_pool(name="w1stage", bufs=2) as w1s:
            for j in range(DT):
                stage = w1s.tile([P, U], F32)
                nc.sync.dma_start(out=stage[:], in_=w1[j * P : (j + 1) * P, :])
                nc.vector.tensor_copy(out=w1t[:, j, :], in_=stage[:])
    else:
        for j in range(DT):
            nc.sync.dma_start(out=w1t[:, j, :], in_=w1[j * P : (j + 1) * P, :])

    # ---- query projection phase (scoped pools; W2 released afterwards) ----
    with (
        tc.tile_pool(name="qphase", bufs=1) as qpool,
        tc.tile_pool(name="qps", bufs=2, space="PSUM") as qps,
    ):
        qnat = qpool.tile([BPC, D], F32R)
        nc.sync.dma_start(out=qnat[:], in_=q[:])
        qT = qpool.tile([P, DT, BPC], F32R)
        for j in range(DT):
            pq = qps.tile([P, BPC], F32R)
            nc.tensor.transpose(
                pq[:], qnat[:, j * P : (j + 1) * P], identr[:BPC, :BPC]
            )
            nc.vector.tensor_copy(out=qT[:, j, :], in_=pq[:])

        w2t = qpool.tile([P, DT, U], F32R)
        for j in range(DT):
            nc.sync.dma_start(out=w2t[:, j, :], in_=w2[j * P : (j + 1) * P, :])

        qpT = qpool.tile([BPC, U], F32)
        for h in range(U // 512):
            pqp = qps.tile([BPC, 512], F32)
            for j in range(DT):
                nc.tensor.matmul(
                    pqp[:],
                    lhsT=qT[:, j, :],
                    rhs=w2t[:, j, h * 512 : (h + 1) * 512],
                    start=(j == 0),
                    stop=(j == DT - 1),
                )
            nc.vector.tensor_copy(out=qpT[:, h * 512 : (h + 1) * 512], in_=pqp[:])

        for i in range(UT):
            pq2 = qps.tile([P, BPC], F32)
            nc.tensor.transpose(
                pq2[:], qpT[:, i * P : (i + 1) * P], identity[:BPC, :BPC]
            )
            nc.scalar.activation(
                out=qp[:, i, :],
                in_=pq2[:],
                func=AF.Identity,
                bias=bias12[:, i : i + 1],
                scale=1.0,
            )

    # ---- main pools ----
    vt_dt = BF16 if kp_bf16 else F32R
    natp = ctx.enter_context(tc.tile_pool(name="nat", bufs=NAT_BUFS))
    vtp = ctx.enter_context(tc.tile_pool(name="vt", bufs=2))
    thp = ctx.enter_context(tc.tile_pool(name="th", bufs=3))
    scp = ctx.enter_context(tc.tile_pool(name="scores", bufs=1))
    awp = ctx.enter_context(tc.tile_pool(name="aw", bufs=1))
    ctxp = ctx.enter_context(tc.tile_pool(name="ctxsb", bufs=1))
    misc = ctx.enter_context(tc.tile_pool(name="misc", bufs=4))

    trps = ctx.enter_context(tc.tile_pool(name="trps", bufs=2, space="PSUM"))
    kpps = ctx.enter_context(tc.tile_pool(name="kpps", bufs=2, space="PSUM"))
    scps = ctx.enter_context(tc.tile_pool(name="scps", bufs=1, space="PSUM"))
    awps = ctx.enter_context(tc.tile_pool(name="awps", bufs=1, space="PSUM"))
    ctps = ctx.enter_context(tc.tile_pool(name="ctps", bufs=2, space="PSUM"))

    for b in range(BPC):
        nats = []
        scores = scp.tile([1, T], F32)
        for ch in range(NCH):
            chunk_nats = []
            for tt in range(TT):
                nat = natp.tile([P, D], F32R)
                t0 = (ch * TT + tt) * P
                nc.sync.dma_start(out=nat[:], in_=v[b, t0 : t0 + P, :])
                chunk_nats.append(nat)
            nats.extend(chunk_nats)

            # transpose the chunk: vT[d=128, j, t=TCH]
            vT = vtp.tile([P, DT, TCH], vt_dt)
            for j in range(DT):
                pt = trps.tile([P, TCH], F32R)
                for tt in range(TT):
                    nc.tensor.transpose(
                        pt[:, tt * P : (tt + 1) * P],
                        chunk_nats[tt][:, j * P : (j + 1) * P],
                        identr[:],
                    )
                nc.vector.tensor_copy(out=vT[:, j, :], in_=pt[:])

            # kp = W1.T @ vT per u-tile; tanh(+bias); score accumulation
            sc = scps.tile([1, TCH], F32)
            for i in range(UT):
                kp = kpps.tile([P, TCH], F32)
                for j in range(DT):
                    nc.tensor.matmul(
                        kp[:],
                        lhsT=w1t[:, j, i * P : (i + 1) * P],
                        rhs=vT[:, j, :],
                        start=(j == 0),
                        stop=(j == DT - 1),
                    )
                th = thp.tile([P, TCH], F32R)
                nc.scalar.activation(
                    out=th[:],
                    in_=kp[:],
                    func=AF.Tanh,
                    bias=qp[:, i, b : b + 1],
                    scale=1.0,
                )
                nc.tensor.matmul(
                    sc[:],
                    lhsT=vsb[:, i : i + 1],
                    rhs=th[:],
                    start=(i == 0),
                    stop=(i == UT - 1),
                    skip_group_check=True,
                )
            nc.vector.tensor_copy(out=scores[:, ch * TCH : (ch + 1) * TCH], in_=sc[:])

        # softmax over [1, T] on partition 0
        negm = misc.tile([1, 1], F32)
        nc.vector.tensor_reduce(
            out=negm[:], in_=scores[:], axis=mybir.AxisListType.X,
            op=mybir.AluOpType.max, negate=True,
        )
        e = awp.tile([1, T], F32)
        ssum = misc.tile([1, 1], F32)
        nc.scalar.activation(
            out=e[:], in_=scores[:], func=AF.Exp, bias=negm[:], scale=1.0,
            accum_out=ssum[:],
        )
        inv = misc.tile([1, 1], F32)
        nc.vector.reciprocal(out=inv[:], in_=ssum[:])
        nc.vector.tensor_scalar_mul(e[:], e[:], inv[:])
        nc.sync.dma_start(out=aw_o[b : b + 1, :], in_=e[:])

        # aw -> awT [t=128, c] via K=1 matmuls against ones
        awT = misc.tile([P, T // P], F32R)
        paw = awps.tile([P, T // P], F32)
        for c in range(T // P):
            nc.tensor.matmul(
                paw[:, c : c + 1],
                lhsT=e[:, c * P : (c + 1) * P],
                rhs=ones[:],
                start=True,
                stop=True,
                skip_group_check=True,
            )
        nc.vector.tensor_copy(out=awT[:], in_=paw[:])

        # ctx[1, d] += awT[:, c].T @ nat_c
        ctx_sb = ctxp.tile([1, D], F32)
        for h in range(D // DCH):
            pc = ctps.tile([1, DCH], F32)
            for c in range(T // P):
                nc.tensor.matmul(
                    pc[:],
                    lhsT=awT[:, c : c + 1],
                    rhs=nats[c][:, h * DCH : (h + 1) * DCH],
                    start=(c == 0),
                    stop=(c == T // P - 1),
                )
            nc.vector.tensor_copy(out=ctx_sb[:, h * DCH : (h + 1) * DCH], in_=pc[:])
        nc.sync.dma_start(out=ctx_o[b : b + 1, :], in_=ctx_sb[:])


def build_kernel():
    nc = bass.Bass("TRN2", target_bir_lowering=False, debug=False)
    q = nc.dram_tensor("query", [BPC, D], F32R, kind="ExternalInput").ap()
    v = nc.dram_tensor("values", [BPC, T, D], F32R, kind="ExternalInput").ap()
    w1 = nc.dram_tensor("W1", [D, U], F32R, kind="ExternalInput").ap()
    b1 = nc.dram_tensor("b1", [U], F32, kind="ExternalInput").ap()
    w2 = nc.dram_tensor("W2", [D, U], F32R, kind="ExternalInput").ap()
    b2 = nc.dram_tensor("b2", [U], F32, kind="ExternalInput").ap()
    vv = nc.dram_tensor("V", [U, 1], F32R, kind="ExternalInput").ap()
    ctx_o = nc.dram_tensor("ctx", [BPC, D], F32, kind="ExternalOutput").ap()
    aw_o = nc.dram_tensor("aw", [BPC, T], F32, kind="ExternalOutput").ap()

    with tile.TileContext(nc) as tc:
        with ExitStack() as ctx:
            _kernel_body(ctx, tc, q, v, w1, b1, w2, b2, vv, ctx_o, aw_o)
    _split_sync_waits(nc)
    return nc


_NC_CACHE = None


def kernel(query, values, W1, b1, W2, b2, V, bv):
    global _NC_CACHE
    query = np.ascontiguousarray(np.asarray(query, dtype=np.float32))
    values = np.ascontiguousarray(np.asarray(values, dtype=np.float32))
    W1 = np.ascontiguousarray(np.asarray(W1, dtype=np.float32))
    b1 = np.ascontiguousarray(np.asarray(b1, dtype=np.float32))
    W2 = np.ascontiguousarray(np.asarray(W2, dtype=np.float32))
    b2 = np.ascontiguousarray(np.asarray(b2, dtype=np.float32))
    V = np.ascontiguousarray(np.asarray(V, dtype=np.float32))

    if _NC_CACHE is None:
        _NC_CACHE = build_kernel()
    nc = _NC_CACHE

    core_ids = list(range(NCORES))
    in_maps = []
    for c in core_ids:
        sl = slice(c * BPC, (c + 1) * BPC)
        in_maps.append(
            {
                "query": query[sl],
                "values": values[sl],
                "W1": W1,
                "b1": b1,
                "W2": W2,
                "b2": b2,
                "V": V,
            }
        )
    res = run_bass_kernel_spmd(nc, in_maps, core_ids)
    ctx_full = np.concatenate([res.results[c]["ctx"] for c in core_ids], axis=0)
    aw_full = np.concatenate([res.results[c]["aw"] for c in core_ids], axis=0)
    return ctx_full.astype(np.float32), aw_full[:, :, None].astype(np.float32)
